# revision 58
# baseline (speedup 1.0000x reference)
"""Trainium2 Bass kernel for gnn_message_passing (nn_Graph_Learn_24739011625001).

Math per batch element n (V=512, F=64):
    xm = x[n, T//2]                                  # [V, F]
    scores[i, j] = sum_f a[f] * |xm[i,f] - xm[j,f]|  # [V, V], symmetric
    tmpS = exp(relu(scores)) = max(exp(scores), 1)
    S[:, j] = tmpS[:, j] / sum_i tmpS[i, j]

Sharding: pure data parallel over N=8 across the 8 NeuronCores (each
core computes one batch element; inputs are prepared/sharded on the
host, outputs gathered and transposed on the host).

Device algorithm (per core) - weighted bf16/fp8 hybrid, ~33.3us in the
CoreSim cost model (4.5x faster than the 151us fp32 baseline),
rel_l2 ~7e-3 (gate 2e-2):
  - The host PRE-WEIGHTS the features: xgw = |a_f| * x, bw = |a_f| * b,
    so the matmul stationary is an exact +-sign(a_f) selector in any
    dtype and no weight quantization ever occurs.
  - Partition p = (j_idx*FG + f_rel): J=32 j's x FG=4 features, G=16
    feature groups, NSET=16 column sets.  One fused tensor_scalar
    (subtract, abs_max) on DVE (bf16 -> 4x mode) / activation(Abs,
    bias, scale=-1) on ACT / tensor_scalar on Pool computes
    a_f*|xm[i,f]-xm[j,f]| for a whole [128, L] tile.
  - TensorE reduces over f, accumulating into PERSISTENT PSUM tiles
    P[t] [128, V] at partition stripe 32*(s%4) (PE tile_position);
    only the triangle i < 32(s+1) is computed (tile 0 square-diag).
    Per (stripe, g-pair) a potential-function balancer picks:
      bf16 mode: 2 bf16 absdiffs (DVE 4x) + 2 bf16 matmuls (1 cyc/row)
      fp8 mode:  2 fp8e4 absdiffs + DoubleRow matmuls (0.5 cyc/row on
                 a 2-deep k-tile = 4x fewer PE cycles per column)
    and an engine per absdiff, minimizing the projected makespan over
    PE/DVE/ACT/Pool with measured per-instruction cost constants.
  - Four g-passes (0:2, 2:4, 4:8, 8:16) so compute starts after an
    eighth of the xgw DMA; dummy warm-up matmuls bring the PE to full
    p-state during the initial DMA wait.
  - Mirror: diag blocks tril-masked during the PSUM->SBUF copy (wedge
    pre-zeroed by DVE memset - GPSIMD may not touch PSUM); upper
    blocks filled by SBUF->SBUF dma_start_transpose (DMA xbar, off the
    compute engines), software-pipelined one tile deep so the ~2.2us
    xbar latency never head-of-line blocks a compute queue.
  - exp on ACT (in-place bf16), max(.,1) + row-sum fused in ONE DVE
    tensor_scalar via accum_out, divide by the per-partition row sum
    (column normalization via symmetry).  The final tile is processed
    block-wise with partial row sums so its postproc tail is short.
    Output rows are S^T; the host transposes.
"""

import sys

if "/opt/trn_rl_repo" not in sys.path:
    sys.path.insert(0, "/opt/trn_rl_repo")

import numpy as np

import concourse.bass as bass
import concourse.tile as tile
from concourse import mybir
from concourse.bass_utils import run_bass_kernel_spmd

N, T, V, F = 8, 8, 512, 64
NCORES = 8
FP32 = mybir.dt.float32
BF16 = mybir.dt.bfloat16
FP8 = mybir.dt.float8e4
NPBF16 = mybir.dt.np(BF16)
NPFP8 = mybir.dt.np(FP8)

J = 32          # j's per set (PSUM stripe base must be a multiple of 32)
FG = 128 // J   # 4 features per partition group
G = F // FG     # 16 feature groups
NPAIR = G // 2  # 8 g-pairs (fp8 DoubleRow k-tiles)
NSET = V // J   # 16 sets
NT = V // 128   # 4 row-tiles of 128
PASSES = [(0, 1), (1, 2), (2, 4), (4, 8)]  # pair-ranges per pass (xg DMA chunking)
NWARM = 26      # PE p-state warm-up matmuls during the DMA wait
MMAX = 256      # max moving cols per DoubleRow matmul (2L <= 512)

# measured cost-model constants (ns) for the engine balancer
COST = {
    "bf16": {"dve": (63.0, 0.22), "act": (185.0, 0.93), "pool": (36.0, 0.70)},
    "fp8": {"dve": (63.0, 0.52), "act": (185.0, 0.93), "pool": (36.0, 0.70)},
}
PE_NS = {"bf16": 0.4167, "fp8": 0.1042}


# fixed postproc work seeds per engine (ns), and the balance target
SEEDS = {"dve": 5800.0, "act": 5100.0, "pool": 700.0, "pe": 3200.0}
MTARGET = 23000.0


def _assignment():
    """(mode, engines) split of the 256 absdiff blocks.

    Per (stripe, g-pair), pick bf16 vs fp8 mode and an engine per
    sub-block by minimizing a convex pressure potential over engine
    loads — this both balances the engines and avoids needlessly
    expensive placements (greedy-makespan tends to inflate total work).
    """
    items = []
    for s in range(NSET):
        # square-diag extent for tile 0 only; triangle elsewhere
        L = 128 if s < NSET // NT else J * (s + 1)
        for k in range(NPAIR):
            items.append((s, k, L))
    items.sort(key=lambda it: -it[2])
    load = {
        "dve": SEEDS["dve"],
        "act": SEEDS["act"],
        "pool": SEEDS["pool"],
        "pe": SEEDS["pe"],
    }
    KPOW = 12

    def phi(ld):
        return sum((v / MTARGET) ** KPOW for v in ld.values())

    engines = ("dve", "act", "pool")
    mode = {}
    eng = {}
    for s, k, L in items:
        best = None
        for m in ("bf16", "fp8"):
            pe_add = 2 * L * PE_NS[m]
            for e1 in engines:
                for e2 in engines:
                    trial = dict(load)
                    trial["pe"] += pe_add
                    trial[e1] += COST[m][e1][0] + COST[m][e1][1] * L
                    trial[e2] += COST[m][e2][0] + COST[m][e2][1] * L
                    p = phi(trial)
                    if best is None or p < best[0] - 1e-12:
                        best = (p, m, e1, e2, trial)
        _, m, e1, e2, trial = best
        mode[(s, k)] = m
        eng[(s, 2 * k)] = e1
        eng[(s, 2 * k + 1)] = e2
        load = trial
    return mode, eng, load, load["pe"]


def _build():
    nc = bass.Bass()
    xg_d = nc.dram_tensor("xg", [128, G, V], BF16, kind="ExternalInput")
    sgb_d = nc.dram_tensor("sgb", [128, G, J], BF16, kind="ExternalInput")
    sg2_d = nc.dram_tensor("sg2", [128, NPAIR, 2, J], FP8, kind="ExternalInput")
    b_d = nc.dram_tensor("bmat", [128, G, NSET], FP32, kind="ExternalInput")
    mi_d = nc.dram_tensor("maskident", [128, 2, 128], BF16, kind="ExternalInput")
    out_d = nc.dram_tensor("out", [V, V], FP32, kind="ExternalOutput")

    mode, eng, _, _ = _assignment()

    with tile.TileContext(nc) as tc:
        with (
            tc.tile_pool(name="singles", bufs=1) as singles,
            tc.tile_pool(name="actb", bufs=16) as actb,
            tc.tile_pool(name="actf", bufs=16) as actf,
            tc.tile_pool(name="obuf", bufs=6) as obuf,
            tc.tile_pool(name="ebuf", bufs=2) as ebuf,
            tc.tile_pool(name="small", bufs=8) as small,
            tc.tile_pool(name="ttmp", bufs=6) as ttmp,
            tc.tile_pool(name="pscore", bufs=1, space="PSUM") as pscore,
            tc.tile_pool(name="pmisc", bufs=2, space="PSUM") as pmisc,
        ):
            xgs = singles.tile([128, G, V], BF16)
            bs = singles.tile([128, G, NSET], FP32)
            mis = singles.tile([128, 2, 128], BF16)
            sgb = singles.tile([128, G, J], BF16)
            sg2 = singles.tile([128, NPAIR, 2, J], FP8)
            # order matters: pass-1 deps first, bigger later chunks last
            nc.sync.dma_start(out=bs, in_=b_d[:, :, :])
            for klo, khi in PASSES:
                if klo == 0:
                    # first chunk split by columns: the first chain's
                    # half-column absdiffs start ~1us earlier
                    nc.sync.dma_start(
                        out=xgs[:, 0 : 2 * khi, 0:MMAX],
                        in_=xg_d[:, 0 : 2 * khi, 0:MMAX],
                    )
                    nc.sync.dma_start(
                        out=xgs[:, 0 : 2 * khi, MMAX:V],
                        in_=xg_d[:, 0 : 2 * khi, MMAX:V],
                    )
                else:
                    nc.sync.dma_start(
                        out=xgs[:, 2 * klo : 2 * khi, :],
                        in_=xg_d[:, 2 * klo : 2 * khi, :],
                    )
                if klo == 0:
                    nc.sync.dma_start(out=mis, in_=mi_d[:, :, :])
                    nc.sync.dma_start(out=sgb, in_=sgb_d[:, :, :])
                    nc.sync.dma_start(out=sg2, in_=sg2_d[:, :, :, :])
            masks = mis[:, 0, :]
            ident = mis[:, 1, :]

            # PE p-state warm-up: dummy zero matmuls while the input DMAs
            # are in flight
            scratch = singles.tile([128, 128], BF16)
            nc.gpsimd.memset(scratch, 0.0)
            for _ in range(NWARM):
                wt = pmisc.tile([128, 128], FP32, tag="wt", name="wt")
                nc.tensor.matmul(wt, scratch, scratch, start=True, stop=True)

            # persistent PSUM score tiles and SBUF assembled-score tiles
            P = []
            Ts = []
            for t in range(NT):
                pt_score = pscore.tile([128, V], FP32, tag=f"p{t}", name=f"p{t}")
                P.append(pt_score)
                ts_tile = singles.tile([128, V], BF16, tag=f"t{t}", name=f"t{t}")
                Ts.append(ts_tile)
                if t > 0:
                    # diag block: triangle matmuls leave an uninit wedge.
                    # GPSIMD cannot touch PSUM on real HW -> DVE memset
                    nc.vector.memset(pt_score[:, 128 * t : 128 * t + 128], 0.0)

            def absdiff(at_ap, s, g, lo, hi):
                e = eng[(s, g)]
                if e == "dve":
                    nc.vector.tensor_scalar(
                        at_ap, xgs[:, g, lo:hi], bs[:, g, s : s + 1], 0.0,
                        op0=mybir.AluOpType.subtract,
                        op1=mybir.AluOpType.abs_max,
                    )
                elif e == "pool":
                    nc.gpsimd.tensor_scalar(
                        at_ap, xgs[:, g, lo:hi], bs[:, g, s : s + 1], 0.0,
                        op0=mybir.AluOpType.subtract,
                        op1=mybir.AluOpType.abs_max,
                    )
                else:
                    nc.scalar.activation(
                        at_ap, xgs[:, g, lo:hi],
                        mybir.ActivationFunctionType.Abs,
                        bias=bs[:, g, s : s + 1], scale=-1.0,
                    )

            def chain(s, klo, khi):
                t = s // (NSET // NT)
                po = J * (s % (NSET // NT))
                # tile 0 is computed square-diag (exact, symmetric, no
                # mirror needed -> short tail); tiles 1..3 triangle-only
                L = 128 if t == 0 else J * (s + 1)
                # the kernel's first chain reads per-column-half so its
                # absdiffs start as soon as the half-column DMA lands
                split = s == NSET - 1 and klo == PASSES[0][0]
                cbs = [(0, MMAX), (MMAX, L)] if (split and L > MMAX) else [(0, L)]
                for k in range(klo, khi):
                    first = k == PASSES[0][0]
                    last = k == NPAIR - 1
                    if mode[(s, k)] == "bf16":
                        for r in range(2):
                            g = 2 * k + r
                            at = actb.tile([128, V], BF16, tag="at")
                            for lo, hi in cbs:
                                absdiff(at[:, lo:hi], s, g, lo, hi)
                                nc.tensor.matmul(
                                    P[t][po : po + J, lo:hi],
                                    sgb[:, g, :], at[:, lo:hi],
                                    start=(first and r == 0 and lo == 0),
                                    stop=(last and r == 1 and hi == L),
                                    tile_position=(0, po),
                                    skip_group_check=True,
                                )
                    else:
                        at2 = actf.tile([128, 2, V], FP8, tag="at2")
                        for r in range(2):
                            for lo, hi in cbs:
                                absdiff(at2[:, r, lo:hi], s, 2 * k + r, lo, hi)
                        nsplit = (L + MMAX - 1) // MMAX
                        for q in range(nsplit):
                            c0, c1 = q * MMAX, min((q + 1) * MMAX, L)
                            nc.tensor.matmul(
                                P[t][po : po + J, c0:c1],
                                sg2[:, k, :, :],
                                at2[:, :, c0:c1],
                                start=(first and q == 0),
                                stop=(last and q == nsplit - 1),
                                perf_mode=mybir.MatmulPerfMode.DoubleRow,
                                tile_position=(0, po),
                                skip_group_check=True,
                            )
                # final tile: plain diag copy per stripe, right after the
                # stripe's chain, so only exp/rowsum is left at the end
                if khi == NPAIR and t == 0:
                    # (GPSIMD may not read PSUM on real HW: DVE/ACT only)
                    if s % 2 == 0:
                        nc.vector.tensor_copy(
                            Ts[0][po : po + J, 0:128], P[0][po : po + J, 0:128]
                        )
                    else:
                        nc.scalar.copy(
                            Ts[0][po : po + J, 0:128], P[0][po : po + J, 0:128]
                        )

            # partial row sums for tile 0, filled block-wise as its upper
            # blocks arrive (so the final tile's postproc tail is short)
            rs4 = singles.tile([128, 4], FP32)

            def expmax_block(tile_idx, c0, acc):
                # exp in place + max(.,1) + row-sum of one 128-col block
                blk = Ts[tile_idx][:, c0 : c0 + 128]
                nc.scalar.activation(blk, blk, mybir.ActivationFunctionType.Exp)
                nc.vector.tensor_scalar(
                    blk, blk, 1.0, None,
                    op0=mybir.AluOpType.max,
                    op1=mybir.AluOpType.add,
                    accum_out=acc,
                )

            # --- two-stage tile pipeline -------------------------------
            # stage A (at the tile's own boundary): PSUM->SBUF copies and
            # all mirror transposes are DISPATCHED.  stage B (one tile
            # later): diag add, exp, max+rowsum, divide, out DMA.  Every
            # cross-engine latency (xbar transpose ~2.2us, sem hops) gets a
            # full tile-chain of slack, so no engine head-of-line blocks.
            dtmps = {}

            def stage_a(t):
                c0 = 128 * t
                # diag: masked copy (kills PSUM garbage + keeps lower tri)
                nc.vector.tensor_tensor(
                    Ts[t][:, c0 : c0 + 128], P[t][:, c0 : c0 + 128], masks,
                    op=mybir.AluOpType.mult,
                )
                # lower off-diag blocks: plain copies off PSUM
                for h in range(t):
                    if h % 2 == 0:
                        nc.vector.tensor_copy(
                            Ts[t][:, 128 * h : 128 * h + 128],
                            P[t][:, 128 * h : 128 * h + 128],
                        )
                    else:
                        nc.scalar.copy(
                            Ts[t][:, 128 * h : 128 * h + 128],
                            P[t][:, 128 * h : 128 * h + 128],
                        )
                # diag mirror transpose via DMA xbar; the add happens in
                # stage B a tile later
                dtmp = ttmp.tile([128, 128], BF16, tag="dtmp")
                nc.sync.dma_start_transpose(dtmp, Ts[t][:, c0 : c0 + 128])
                dtmps[t] = dtmp
                # provide upper blocks for tiles processed later (t' < t)
                for tp in range(1, t):
                    nc.sync.dma_start_transpose(
                        Ts[tp][:, c0 : c0 + 128],
                        Ts[t][:, 128 * tp : 128 * tp + 128],
                    )
                if t >= 2:
                    nc.sync.dma_start_transpose(
                        Ts[0][:, c0 : c0 + 128], Ts[t][:, 0:128]
                    )
                elif t == 1:
                    # no slack left for the xbar: PE transpose (~150ns)
                    pt0 = pmisc.tile([128, 128], BF16, tag="pt")
                    nc.tensor.transpose(pt0, Ts[1][:, 0:128], ident)
                    nc.scalar.copy(Ts[0][:, c0 : c0 + 128], pt0)
                    expmax_block(0, c0, rs4[:, 1:2])

            def stage_b(t):
                c0 = 128 * t
                nc.vector.tensor_tensor(
                    Ts[t][:, c0 : c0 + 128], Ts[t][:, c0 : c0 + 128],
                    dtmps.pop(t),
                    op=mybir.AluOpType.add,
                )
                rs = small.tile([128, 1], FP32, tag="rs")
                nc.scalar.activation(
                    Ts[t], Ts[t], mybir.ActivationFunctionType.Exp
                )
                nc.vector.tensor_scalar(
                    Ts[t], Ts[t], 1.0, None,
                    op0=mybir.AluOpType.max,
                    op1=mybir.AluOpType.add,
                    accum_out=rs,
                )
                ob = obuf.tile([128, V], FP32, tag="ob")
                nc.vector.tensor_scalar(
                    ob, Ts[t], rs, None, op0=mybir.AluOpType.divide
                )
                # t=1's out DMA goes via ACT so the final tile's DMA never
                # queues behind it on SP
                if t == 1:
                    nc.scalar.dma_start(out=out_d[c0 : c0 + 128, :], in_=ob)
                else:
                    nc.sync.dma_start(out=out_d[c0 : c0 + 128, :], in_=ob)

            def process_final():
                # tile 0 finale: exp of the (already exact) diag block,
                # partial-sum combine, divide, out
                expmax_block(0, 0, rs4[:, 0:1])
                rs = small.tile([128, 1], FP32, tag="rs")
                nc.vector.reduce_sum(rs, rs4, axis=mybir.AxisListType.X)
                ob = obuf.tile([128, V], FP32, tag="ob")
                nc.vector.tensor_scalar(
                    ob, Ts[0], rs, None, op0=mybir.AluOpType.divide
                )
                nc.sync.dma_start(out=out_d[0:128, :], in_=ob)

            for pi, (klo, khi) in enumerate(PASSES):
                last = pi == len(PASSES) - 1
                # early passes run ascending (tiny chains first while the
                # xg DMA chunks stream in); the last pass descending so the
                # tile postproc pipeline works t=3..0
                order = range(NSET - 1, -1, -1) if last else range(NSET)
                for s in order:
                    chain(s, klo, khi)
                    if not last:
                        continue
                    # boundary slots (one-chain deferral baked in):
                    if s == 11:
                        stage_a(3)
                    elif s == 7:
                        stage_a(2)
                        stage_b(3)
                    elif s == 4:
                        expmax_block(0, 384, rs4[:, 3:4])
                    elif s == 3:
                        stage_a(1)
                        stage_b(2)
                    elif s == 2:
                        expmax_block(0, 256, rs4[:, 2:3])
                        stage_b(1)
            process_final()
    return nc


# revision 77
# speedup vs baseline: 1.0078x; 1.0078x over previous
"""Trainium2 Bass kernel for gnn_message_passing (nn_Graph_Learn_24739011625001).

Math per batch element n (V=512, F=64):
    xm = x[n, T//2]                                  # [V, F]
    scores[i, j] = sum_f a[f] * |xm[i,f] - xm[j,f]|  # [V, V], symmetric
    tmpS = exp(relu(scores)) = max(exp(scores), 1)
    S[:, j] = tmpS[:, j] / sum_i tmpS[i, j]

Sharding: pure data parallel over N=8 across the 8 NeuronCores (each
core computes one batch element; inputs are prepared/sharded on the
host, outputs gathered and transposed on the host).

Device algorithm (per core) - weighted bf16/fp8 hybrid, ~32.8us in the
CoreSim cost model (4.6x faster than the 151us fp32 baseline),
rel_l2 ~7e-3 (gate 2e-2):
  - The host PRE-WEIGHTS the features: xgw = |a_f| * x, bw = |a_f| * b,
    so the matmul stationary is an exact +-sign(a_f) selector in any
    dtype and no weight quantization ever occurs.
  - Partition p = (j_idx*FG + f_rel): J=32 j's x FG=4 features, G=16
    feature groups, NSET=16 column sets.  One fused tensor_scalar
    (subtract, abs_max) on DVE (bf16 -> 4x mode) / activation(Abs,
    bias, scale=-1) on ACT / tensor_scalar on Pool computes
    a_f*|xm[i,f]-xm[j,f]| for a whole [128, L] tile.
  - TensorE reduces over f, accumulating into PERSISTENT PSUM tiles
    P[t] [128, V] at partition stripe 32*(s%4) (PE tile_position);
    only the triangle i < 32(s+1) is computed (tile 0 square-diag).
    Per (stripe, g-pair) a potential-function balancer picks:
      bf16 mode: 2 bf16 absdiffs (DVE 4x) + 2 bf16 matmuls (1 cyc/row)
      fp8 mode:  2 fp8e4 absdiffs + DoubleRow matmuls (0.5 cyc/row on
                 a 2-deep k-tile = 4x fewer PE cycles per column)
    and an engine per absdiff, minimizing the projected makespan over
    PE/DVE/ACT/Pool with measured per-instruction cost constants.
  - Four g-passes (0:2, 2:4, 4:8, 8:16) so compute starts after an
    eighth of the xgw DMA; dummy warm-up matmuls bring the PE to full
    p-state during the initial DMA wait.
  - Mirror: diag blocks tril-masked during the PSUM->SBUF copy (wedge
    pre-zeroed by DVE memset - GPSIMD may not touch PSUM); upper
    blocks filled by SBUF->SBUF dma_start_transpose (DMA xbar, off the
    compute engines), software-pipelined one tile deep so the ~2.2us
    xbar latency never head-of-line blocks a compute queue.
  - exp on ACT (in-place bf16), max(.,1) + row-sum fused in ONE DVE
    tensor_scalar via accum_out, divide by the per-partition row sum
    (column normalization via symmetry).  The final tile is processed
    block-wise with partial row sums so its postproc tail is short.
    Output rows are S^T; the host transposes.
"""

import sys

if "/opt/trn_rl_repo" not in sys.path:
    sys.path.insert(0, "/opt/trn_rl_repo")

import numpy as np

import concourse.bass as bass
import concourse.tile as tile
from concourse import mybir
from concourse.bass_utils import run_bass_kernel_spmd

N, T, V, F = 8, 8, 512, 64
NCORES = 8
FP32 = mybir.dt.float32
BF16 = mybir.dt.bfloat16
FP8 = mybir.dt.float8e4
NPBF16 = mybir.dt.np(BF16)
NPFP8 = mybir.dt.np(FP8)

J = 32          # j's per set (PSUM stripe base must be a multiple of 32)
FG = 128 // J   # 4 features per partition group
G = F // FG     # 16 feature groups
NPAIR = G // 2  # 8 g-pairs (fp8 DoubleRow k-tiles)
NSET = V // J   # 16 sets
NT = V // 128   # 4 row-tiles of 128
PASSES = [(0, 1), (1, 2), (2, 4), (4, 8)]  # pair-ranges per pass (xg DMA chunking)
NWARM = 26      # PE p-state warm-up matmuls during the DMA wait
MMAX = 256      # max moving cols per DoubleRow matmul (2L <= 512)

# measured cost-model constants (ns) for the engine balancer
COST = {
    "bf16": {"dve": (63.0, 0.22), "act": (185.0, 0.93), "pool": (36.0, 0.70)},
    "fp8": {"dve": (63.0, 0.52), "act": (185.0, 0.93), "pool": (36.0, 0.70)},
}
PE_NS = {"bf16": 0.4167, "fp8": 0.1042}


# fixed postproc work seeds per engine (ns), and the balance target
SEEDS = {"dve": 5800.0, "act": 5100.0, "pool": 700.0, "pe": 3200.0}
MTARGET = 23000.0


def _assignment():
    """(mode, engines) split of the 256 absdiff blocks.

    Per (stripe, g-pair), pick bf16 vs fp8 mode and an engine per
    sub-block by minimizing a convex pressure potential over engine
    loads — this both balances the engines and avoids needlessly
    expensive placements (greedy-makespan tends to inflate total work).
    """
    items = []
    for s in range(NSET):
        # square-diag extent for tile 0 only; triangle elsewhere
        L = 128 if s < NSET // NT else J * (s + 1)
        for k in range(NPAIR):
            items.append((s, k, L))
    items.sort(key=lambda it: -it[2])
    load = {
        "dve": SEEDS["dve"],
        "act": SEEDS["act"],
        "pool": SEEDS["pool"],
        "pe": SEEDS["pe"],
    }
    KPOW = 12

    def phi(ld):
        return sum((v / MTARGET) ** KPOW for v in ld.values())

    engines = ("dve", "act", "pool")
    mode = {}
    eng = {}
    for s, k, L in items:
        best = None
        for m in ("bf16", "fp8"):
            pe_add = 2 * L * PE_NS[m]
            for e1 in engines:
                for e2 in engines:
                    trial = dict(load)
                    trial["pe"] += pe_add
                    trial[e1] += COST[m][e1][0] + COST[m][e1][1] * L
                    trial[e2] += COST[m][e2][0] + COST[m][e2][1] * L
                    p = phi(trial)
                    if best is None or p < best[0] - 1e-12:
                        best = (p, m, e1, e2, trial)
        _, m, e1, e2, trial = best
        mode[(s, k)] = m
        eng[(s, 2 * k)] = e1
        eng[(s, 2 * k + 1)] = e2
        load = trial
    return mode, eng, load, load["pe"]


def _build():
    nc = bass.Bass()
    xg_d = nc.dram_tensor("xg", [128, G, V], BF16, kind="ExternalInput")
    sgb_d = nc.dram_tensor("sgb", [128, G, J], BF16, kind="ExternalInput")
    sg2_d = nc.dram_tensor("sg2", [128, NPAIR, 2, J], FP8, kind="ExternalInput")
    b_d = nc.dram_tensor("bmat", [128, G, NSET], FP32, kind="ExternalInput")
    mi_d = nc.dram_tensor("maskident", [128, 2, 128], BF16, kind="ExternalInput")
    out_d = nc.dram_tensor("out", [V, V], FP32, kind="ExternalOutput")

    mode, eng, _, _ = _assignment()

    with tile.TileContext(nc) as tc:
        with (
            tc.tile_pool(name="singles", bufs=1) as singles,
            tc.tile_pool(name="actb", bufs=24) as actb,
            tc.tile_pool(name="actf", bufs=16) as actf,
            tc.tile_pool(name="obuf", bufs=6) as obuf,
            tc.tile_pool(name="ebuf", bufs=2) as ebuf,
            tc.tile_pool(name="small", bufs=8) as small,
            tc.tile_pool(name="ttmp", bufs=6) as ttmp,
            tc.tile_pool(name="pscore", bufs=1, space="PSUM") as pscore,
            tc.tile_pool(name="pmisc", bufs=2, space="PSUM") as pmisc,
        ):
            xgs = singles.tile([128, G, V], BF16)
            bs = singles.tile([128, G, NSET], FP32)
            mis = singles.tile([128, 2, 128], BF16)
            sgb = singles.tile([128, G, J], BF16)
            sg2 = singles.tile([128, NPAIR, 2, J], FP8)
            # order matters: pass-1 deps first, bigger later chunks last
            nc.sync.dma_start(out=bs, in_=b_d[:, :, :])
            for klo, khi in PASSES:
                if klo == 0:
                    # first chunk split by columns: the first chain's
                    # half-column absdiffs start ~1us earlier
                    nc.sync.dma_start(
                        out=xgs[:, 0 : 2 * khi, 0:MMAX],
                        in_=xg_d[:, 0 : 2 * khi, 0:MMAX],
                    )
                    nc.sync.dma_start(
                        out=xgs[:, 0 : 2 * khi, MMAX:V],
                        in_=xg_d[:, 0 : 2 * khi, MMAX:V],
                    )
                else:
                    nc.sync.dma_start(
                        out=xgs[:, 2 * klo : 2 * khi, :],
                        in_=xg_d[:, 2 * klo : 2 * khi, :],
                    )
                if klo == 0:
                    nc.sync.dma_start(out=mis, in_=mi_d[:, :, :])
                    nc.sync.dma_start(out=sgb, in_=sgb_d[:, :, :])
                    nc.sync.dma_start(out=sg2, in_=sg2_d[:, :, :, :])
            masks = mis[:, 0, :]
            ident = mis[:, 1, :]

            # PE p-state warm-up: dummy zero matmuls while the input DMAs
            # are in flight
            scratch = singles.tile([128, 128], BF16)
            nc.gpsimd.memset(scratch, 0.0)
            for _ in range(NWARM):
                wt = pmisc.tile([128, 128], FP32, tag="wt", name="wt")
                nc.tensor.matmul(wt, scratch, scratch, start=True, stop=True)

            # persistent PSUM score tiles and SBUF assembled-score tiles
            P = []
            Ts = []
            for t in range(NT):
                pt_score = pscore.tile([128, V], FP32, tag=f"p{t}", name=f"p{t}")
                P.append(pt_score)
                ts_tile = singles.tile([128, V], BF16, tag=f"t{t}", name=f"t{t}")
                Ts.append(ts_tile)
                if t > 0:
                    # diag block: triangle matmuls leave an uninit wedge.
                    # GPSIMD cannot touch PSUM on real HW; ACT is idle at
                    # kernel start -> scalar memzero
                    nc.scalar.memzero(pt_score[:, 128 * t : 128 * t + 128])

            def absdiff(at_ap, s, g, lo, hi):
                e = eng[(s, g)]
                if e == "dve":
                    nc.vector.tensor_scalar(
                        at_ap, xgs[:, g, lo:hi], bs[:, g, s : s + 1], 0.0,
                        op0=mybir.AluOpType.subtract,
                        op1=mybir.AluOpType.abs_max,
                    )
                elif e == "pool":
                    nc.gpsimd.tensor_scalar(
                        at_ap, xgs[:, g, lo:hi], bs[:, g, s : s + 1], 0.0,
                        op0=mybir.AluOpType.subtract,
                        op1=mybir.AluOpType.abs_max,
                    )
                else:
                    nc.scalar.activation(
                        at_ap, xgs[:, g, lo:hi],
                        mybir.ActivationFunctionType.Abs,
                        bias=bs[:, g, s : s + 1], scale=-1.0,
                    )

            def chain(s, klo, khi):
                t = s // (NSET // NT)
                po = J * (s % (NSET // NT))
                # tile 0 is computed square-diag (exact, symmetric, no
                # mirror needed -> short tail); tiles 1..3 triangle-only
                L = 128 if t == 0 else J * (s + 1)
                # the kernel's first chain reads per-column-half so its
                # absdiffs start as soon as the half-column DMA lands
                split = s == NSET - 1 and klo == PASSES[0][0]
                cbs = [(0, MMAX), (MMAX, L)] if (split and L > MMAX) else [(0, L)]
                for k in range(klo, khi):
                    first = k == PASSES[0][0]
                    last = k == NPAIR - 1
                    if mode[(s, k)] == "bf16":
                        for r in range(2):
                            g = 2 * k + r
                            at = actb.tile([128, V], BF16, tag="at")
                            for lo, hi in cbs:
                                absdiff(at[:, lo:hi], s, g, lo, hi)
                                nc.tensor.matmul(
                                    P[t][po : po + J, lo:hi],
                                    sgb[:, g, :], at[:, lo:hi],
                                    start=(first and r == 0 and lo == 0),
                                    stop=(last and r == 1 and hi == L),
                                    tile_position=(0, po),
                                    skip_group_check=True,
                                )
                    else:
                        at2 = actf.tile([128, 2, V], FP8, tag="at2")
                        for r in range(2):
                            for lo, hi in cbs:
                                absdiff(at2[:, r, lo:hi], s, 2 * k + r, lo, hi)
                        nsplit = (L + MMAX - 1) // MMAX
                        for q in range(nsplit):
                            c0, c1 = q * MMAX, min((q + 1) * MMAX, L)
                            nc.tensor.matmul(
                                P[t][po : po + J, c0:c1],
                                sg2[:, k, :, :],
                                at2[:, :, c0:c1],
                                start=(first and q == 0),
                                stop=(last and q == nsplit - 1),
                                perf_mode=mybir.MatmulPerfMode.DoubleRow,
                                tile_position=(0, po),
                                skip_group_check=True,
                            )
                # final tile: plain diag copy per stripe, right after the
                # stripe's chain, so only exp/rowsum is left at the end
                if khi == NPAIR and t == 0:
                    # (GPSIMD may not read PSUM on real HW: DVE/ACT only)
                    if s % 2 == 0:
                        nc.vector.tensor_copy(
                            Ts[0][po : po + J, 0:128], P[0][po : po + J, 0:128]
                        )
                    else:
                        nc.scalar.copy(
                            Ts[0][po : po + J, 0:128], P[0][po : po + J, 0:128]
                        )

            # partial row sums for tile 0, filled block-wise as its upper
            # blocks arrive (so the final tile's postproc tail is short)
            rs4 = singles.tile([128, 4], FP32)

            def expmax_block(tile_idx, c0, acc):
                # exp in place + max(.,1) + row-sum of one 128-col block
                blk = Ts[tile_idx][:, c0 : c0 + 128]
                nc.scalar.activation(blk, blk, mybir.ActivationFunctionType.Exp)
                nc.vector.tensor_scalar(
                    blk, blk, 1.0, None,
                    op0=mybir.AluOpType.max,
                    op1=mybir.AluOpType.add,
                    accum_out=acc,
                )

            # --- two-stage tile pipeline -------------------------------
            # stage A (at the tile's own boundary): PSUM->SBUF copies and
            # all mirror transposes are DISPATCHED.  stage B (one tile
            # later): diag add, exp, max+rowsum, divide, out DMA.  Every
            # cross-engine latency (xbar transpose ~2.2us, sem hops) gets a
            # full tile-chain of slack, so no engine head-of-line blocks.
            dtmps = {}

            def stage_a(t):
                c0 = 128 * t
                # diag: masked copy (kills PSUM garbage + keeps lower tri)
                nc.vector.tensor_tensor(
                    Ts[t][:, c0 : c0 + 128], P[t][:, c0 : c0 + 128], masks,
                    op=mybir.AluOpType.mult,
                )
                # lower off-diag blocks: plain copies off PSUM, all on
                # ACT (it has slack; DVE is the critical engine)
                for h in range(t):
                    nc.scalar.copy(
                        Ts[t][:, 128 * h : 128 * h + 128],
                        P[t][:, 128 * h : 128 * h + 128],
                    )
                # diag mirror transpose via DMA xbar; the add happens in
                # stage B a tile later
                dtmp = ttmp.tile([128, 128], BF16, tag="dtmp")
                nc.sync.dma_start_transpose(dtmp, Ts[t][:, c0 : c0 + 128])
                dtmps[t] = dtmp
                # provide upper blocks for tiles processed later (t' < t)
                for tp in range(1, t):
                    nc.sync.dma_start_transpose(
                        Ts[tp][:, c0 : c0 + 128],
                        Ts[t][:, 128 * tp : 128 * tp + 128],
                    )
                if t >= 2:
                    nc.sync.dma_start_transpose(
                        Ts[0][:, c0 : c0 + 128], Ts[t][:, 0:128]
                    )
                elif t == 1:
                    # no slack left for the xbar: PE transpose (~150ns)
                    pt0 = pmisc.tile([128, 128], BF16, tag="pt")
                    nc.tensor.transpose(pt0, Ts[1][:, 0:128], ident)
                    nc.scalar.copy(Ts[0][:, c0 : c0 + 128], pt0)
                    expmax_block(0, c0, rs4[:, 1:2])

            def stage_b(t):
                c0 = 128 * t
                nc.vector.tensor_tensor(
                    Ts[t][:, c0 : c0 + 128], Ts[t][:, c0 : c0 + 128],
                    dtmps.pop(t),
                    op=mybir.AluOpType.add,
                )
                rs = small.tile([128, 1], FP32, tag="rs")
                nc.scalar.activation(
                    Ts[t], Ts[t], mybir.ActivationFunctionType.Exp
                )
                nc.vector.tensor_scalar(
                    Ts[t], Ts[t], 1.0, None,
                    op0=mybir.AluOpType.max,
                    op1=mybir.AluOpType.add,
                    accum_out=rs,
                )
                ob = obuf.tile([128, V], FP32, tag="ob")
                nc.gpsimd.tensor_scalar(
                    ob, Ts[t], rs, None, op0=mybir.AluOpType.divide
                )
                # t=1's out DMA goes via ACT so the final tile's DMA never
                # queues behind it on SP
                if t == 1:
                    nc.scalar.dma_start(out=out_d[c0 : c0 + 128, :], in_=ob)
                else:
                    nc.sync.dma_start(out=out_d[c0 : c0 + 128, :], in_=ob)

            def process_final():
                # tile 0 finale: exp of the (already exact) diag block,
                # partial-sum combine, divide, out
                expmax_block(0, 0, rs4[:, 0:1])
                rs = small.tile([128, 1], FP32, tag="rs")
                nc.vector.reduce_sum(rs, rs4, axis=mybir.AxisListType.X)
                ob = obuf.tile([128, V], FP32, tag="ob")
                nc.vector.tensor_scalar(
                    ob, Ts[0], rs, None, op0=mybir.AluOpType.divide
                )
                nc.sync.dma_start(out=out_d[0:128, :], in_=ob)

            for pi, (klo, khi) in enumerate(PASSES):
                last = pi == len(PASSES) - 1
                # early passes run ascending (tiny chains first while the
                # xg DMA chunks stream in); the last pass descending so the
                # tile postproc pipeline works t=3..0
                order = range(NSET - 1, -1, -1) if last else range(NSET)
                for s in order:
                    chain(s, klo, khi)
                    if not last:
                        continue
                    # boundary slots (one-chain deferral baked in):
                    if s == 11:
                        stage_a(3)
                    elif s == 7:
                        stage_a(2)
                        stage_b(3)
                    elif s == 4:
                        expmax_block(0, 384, rs4[:, 3:4])
                    elif s == 3:
                        stage_a(1)
                        stage_b(2)
                    elif s == 2:
                        expmax_block(0, 256, rs4[:, 2:3])
                        stage_b(1)
            process_final()
    return nc


# revision 89
# speedup vs baseline: 1.0206x; 1.0126x over previous
"""Trainium2 Bass kernel for gnn_message_passing (nn_Graph_Learn_24739011625001).

Math per batch element n (V=512, F=64):
    xm = x[n, T//2]                                  # [V, F]
    scores[i, j] = sum_f a[f] * |xm[i,f] - xm[j,f]|  # [V, V], symmetric
    tmpS = exp(relu(scores)) = max(exp(scores), 1)
    S[:, j] = tmpS[:, j] / sum_i tmpS[i, j]

Sharding: pure data parallel over N=8 across the 8 NeuronCores (each
core computes one batch element; inputs are prepared/sharded on the
host, outputs gathered and transposed on the host).

Device algorithm (per core) - weighted bf16/fp8 hybrid, ~32.8us in the
CoreSim cost model (4.6x faster than the 151us fp32 baseline),
rel_l2 ~7e-3 (gate 2e-2):
  - The host PRE-WEIGHTS the features: xgw = |a_f| * x, bw = |a_f| * b,
    so the matmul stationary is an exact +-sign(a_f) selector in any
    dtype and no weight quantization ever occurs.
  - Partition p = (j_idx*FG + f_rel): J=32 j's x FG=4 features, G=16
    feature groups, NSET=16 column sets.  One fused tensor_scalar
    (subtract, abs_max) on DVE (bf16 -> 4x mode) / activation(Abs,
    bias, scale=-1) on ACT / tensor_scalar on Pool computes
    a_f*|xm[i,f]-xm[j,f]| for a whole [128, L] tile.
  - TensorE reduces over f, accumulating into PERSISTENT PSUM tiles
    P[t] [128, V] at partition stripe 32*(s%4) (PE tile_position);
    only the triangle i < 32(s+1) is computed (tile 0 square-diag).
    Per (stripe, g-pair) a potential-function balancer picks:
      bf16 mode: 2 bf16 absdiffs (DVE 4x) + 2 bf16 matmuls (1 cyc/row)
      fp8 mode:  2 fp8e4 absdiffs + DoubleRow matmuls (0.5 cyc/row on
                 a 2-deep k-tile = 4x fewer PE cycles per column)
    and an engine per absdiff, minimizing the projected makespan over
    PE/DVE/ACT/Pool with measured per-instruction cost constants.
  - Four g-passes (0:2, 2:4, 4:8, 8:16) so compute starts after an
    eighth of the xgw DMA; dummy warm-up matmuls bring the PE to full
    p-state during the initial DMA wait.
  - Mirror: diag blocks tril-masked during the PSUM->SBUF copy (wedge
    pre-zeroed by DVE memset - GPSIMD may not touch PSUM); upper
    blocks filled by SBUF->SBUF dma_start_transpose (DMA xbar, off the
    compute engines), software-pipelined one tile deep so the ~2.2us
    xbar latency never head-of-line blocks a compute queue.
  - exp on ACT (in-place bf16), max(.,1) + row-sum fused in ONE DVE
    tensor_scalar via accum_out, divide by the per-partition row sum
    (column normalization via symmetry).  The final tile is processed
    block-wise with partial row sums so its postproc tail is short.
    Output rows are S^T; the host transposes.
"""

import sys

if "/opt/trn_rl_repo" not in sys.path:
    sys.path.insert(0, "/opt/trn_rl_repo")

import numpy as np

import concourse.bass as bass
import concourse.tile as tile
from concourse import mybir
from concourse.bass_utils import run_bass_kernel_spmd

N, T, V, F = 8, 8, 512, 64
NCORES = 8
FP32 = mybir.dt.float32
BF16 = mybir.dt.bfloat16
FP8 = mybir.dt.float8e4
NPBF16 = mybir.dt.np(BF16)
NPFP8 = mybir.dt.np(FP8)

J = 32          # j's per set (PSUM stripe base must be a multiple of 32)
FG = 128 // J   # 4 features per partition group
G = F // FG     # 16 feature groups
NPAIR = G // 2  # 8 g-pairs (fp8 DoubleRow k-tiles)
NSET = V // J   # 16 sets
NT = V // 128   # 4 row-tiles of 128
PASSES = [(0, 1), (1, 2), (2, 4), (4, 8)]  # pair-ranges per pass (xg DMA chunking)
NWARM = 26      # PE p-state warm-up matmuls during the DMA wait
MMAX = 256      # max moving cols per DoubleRow matmul (2L <= 512)

# measured cost-model constants (ns) for the engine balancer
COST = {
    "bf16": {"dve": (63.0, 0.22), "act": (185.0, 0.93), "pool": (36.0, 0.70)},
    "fp8": {"dve": (63.0, 0.52), "act": (185.0, 0.93), "pool": (36.0, 0.70)},
}
PE_NS = {"bf16": 0.4167, "fp8": 0.1042}


# fixed postproc work seeds per engine (ns), and the balance target
SEEDS = {"dve": 5800.0, "act": 5100.0, "pool": 700.0, "pe": 3200.0}
MTARGET = 23000.0


def _assignment():
    """(mode, engines) split of the 256 absdiff blocks.

    Per (stripe, g-pair), pick bf16 vs fp8 mode and an engine per
    sub-block by minimizing a convex pressure potential over engine
    loads — this both balances the engines and avoids needlessly
    expensive placements (greedy-makespan tends to inflate total work).
    """
    items = []
    for s in range(NSET):
        # square-diag extent for tile 0 only; triangle elsewhere
        L = 128 if s < NSET // NT else J * (s + 1)
        for k in range(NPAIR):
            items.append((s, k, L))
    items.sort(key=lambda it: -it[2])
    load = {
        "dve": SEEDS["dve"],
        "act": SEEDS["act"],
        "pool": SEEDS["pool"],
        "pe": SEEDS["pe"],
    }
    KPOW = 12

    def phi(ld):
        return sum((v / MTARGET) ** KPOW for v in ld.values())

    engines = ("dve", "act", "pool")
    mode = {}
    eng = {}
    for s, k, L in items:
        best = None
        for m in ("bf16", "fp8"):
            pe_add = 2 * L * PE_NS[m]
            for e1 in engines:
                for e2 in engines:
                    trial = dict(load)
                    trial["pe"] += pe_add
                    trial[e1] += COST[m][e1][0] + COST[m][e1][1] * L
                    trial[e2] += COST[m][e2][0] + COST[m][e2][1] * L
                    p = phi(trial)
                    if best is None or p < best[0] - 1e-12:
                        best = (p, m, e1, e2, trial)
        _, m, e1, e2, trial = best
        mode[(s, k)] = m
        eng[(s, 2 * k)] = e1
        eng[(s, 2 * k + 1)] = e2
        load = trial
    return mode, eng, load, load["pe"]


def _build():
    nc = bass.Bass()
    xg_d = nc.dram_tensor("xg", [128, G, V], BF16, kind="ExternalInput")
    sgb_d = nc.dram_tensor("sgb", [128, G, J], BF16, kind="ExternalInput")
    sg2_d = nc.dram_tensor("sg2", [128, NPAIR, 2, J], FP8, kind="ExternalInput")
    b_d = nc.dram_tensor("bmat", [128, G, NSET], FP32, kind="ExternalInput")
    mi_d = nc.dram_tensor("maskident", [128, 2, 128], BF16, kind="ExternalInput")
    out_d = nc.dram_tensor("out", [V, V], FP32, kind="ExternalOutput")

    mode, eng, _, _ = _assignment()

    with tile.TileContext(nc) as tc:
        with (
            tc.tile_pool(name="singles", bufs=1) as singles,
            tc.tile_pool(name="actb", bufs=26) as actb,
            tc.tile_pool(name="actf", bufs=16) as actf,
            tc.tile_pool(name="obuf", bufs=6) as obuf,
            tc.tile_pool(name="ebuf", bufs=2) as ebuf,
            tc.tile_pool(name="small", bufs=8) as small,
            tc.tile_pool(name="ttmp", bufs=6) as ttmp,
            tc.tile_pool(name="pscore", bufs=1, space="PSUM") as pscore,
            tc.tile_pool(name="pmisc", bufs=2, space="PSUM") as pmisc,
        ):
            xgs = singles.tile([128, G, V], BF16)
            bs = singles.tile([128, G, NSET], FP32)
            mis = singles.tile([128, 2, 128], BF16)
            sgb = singles.tile([128, G, J], BF16)
            sg2 = singles.tile([128, NPAIR, 2, J], FP8)
            # order matters: pass-1 deps first, bigger later chunks last
            nc.sync.dma_start(out=bs, in_=b_d[:, :, :])
            for klo, khi in PASSES:
                if klo == 0:
                    # first chunk split by columns: the first chain's
                    # half-column absdiffs start ~1us earlier
                    nc.sync.dma_start(
                        out=xgs[:, 0 : 2 * khi, 0:MMAX],
                        in_=xg_d[:, 0 : 2 * khi, 0:MMAX],
                    )
                    nc.sync.dma_start(
                        out=xgs[:, 0 : 2 * khi, MMAX:V],
                        in_=xg_d[:, 0 : 2 * khi, MMAX:V],
                    )
                else:
                    nc.sync.dma_start(
                        out=xgs[:, 2 * klo : 2 * khi, :],
                        in_=xg_d[:, 2 * klo : 2 * khi, :],
                    )
                if klo == 0:
                    nc.sync.dma_start(out=mis, in_=mi_d[:, :, :])
                    nc.sync.dma_start(out=sgb, in_=sgb_d[:, :, :])
                    nc.sync.dma_start(out=sg2, in_=sg2_d[:, :, :, :])
            masks = mis[:, 0, :]
            ident = mis[:, 1, :]

            # PE p-state warm-up: dummy zero matmuls while the input DMAs
            # are in flight
            scratch = singles.tile([128, 128], BF16)
            nc.gpsimd.memset(scratch, 0.0)
            for _ in range(NWARM):
                wt = pmisc.tile([128, 128], FP32, tag="wt", name="wt")
                nc.tensor.matmul(wt, scratch, scratch, start=True, stop=True)

            # persistent PSUM score tiles and SBUF assembled-score tiles
            P = []
            Ts = []
            for t in range(NT):
                pt_score = pscore.tile([128, V], FP32, tag=f"p{t}", name=f"p{t}")
                P.append(pt_score)
                ts_tile = singles.tile([128, V], BF16, tag=f"t{t}", name=f"t{t}")
                Ts.append(ts_tile)
                if t > 0:
                    # diag block: triangle matmuls leave an uninit wedge.
                    # GPSIMD cannot touch PSUM on real HW; ACT is idle at
                    # kernel start -> scalar memzero
                    nc.scalar.memzero(pt_score[:, 128 * t : 128 * t + 128])

            def absdiff(at_ap, s, g, lo, hi):
                e = eng[(s, g)]
                if e == "dve":
                    nc.vector.tensor_scalar(
                        at_ap, xgs[:, g, lo:hi], bs[:, g, s : s + 1], 0.0,
                        op0=mybir.AluOpType.subtract,
                        op1=mybir.AluOpType.abs_max,
                    )
                elif e == "pool":
                    nc.gpsimd.tensor_scalar(
                        at_ap, xgs[:, g, lo:hi], bs[:, g, s : s + 1], 0.0,
                        op0=mybir.AluOpType.subtract,
                        op1=mybir.AluOpType.abs_max,
                    )
                else:
                    nc.scalar.activation(
                        at_ap, xgs[:, g, lo:hi],
                        mybir.ActivationFunctionType.Abs,
                        bias=bs[:, g, s : s + 1], scale=-1.0,
                    )

            def chain(s, klo, khi):
                t = s // (NSET // NT)
                po = J * (s % (NSET // NT))
                # tile 0 is computed square-diag (exact, symmetric, no
                # mirror needed -> short tail); tiles 1..3 triangle-only
                L = 128 if t == 0 else J * (s + 1)
                # the kernel's first chain reads per-column-half so its
                # absdiffs start as soon as the half-column DMA lands
                split = s == NSET - 1 and klo == PASSES[0][0]
                cbs = [(0, MMAX), (MMAX, L)] if (split and L > MMAX) else [(0, L)]
                for k in range(klo, khi):
                    first = k == PASSES[0][0]
                    last = k == NPAIR - 1
                    if mode[(s, k)] == "bf16":
                        for r in range(2):
                            g = 2 * k + r
                            at = actb.tile([128, V], BF16, tag="at")
                            for lo, hi in cbs:
                                absdiff(at[:, lo:hi], s, g, lo, hi)
                                nc.tensor.matmul(
                                    P[t][po : po + J, lo:hi],
                                    sgb[:, g, :], at[:, lo:hi],
                                    start=(first and r == 0 and lo == 0),
                                    stop=(last and r == 1 and hi == L),
                                    tile_position=(0, po),
                                    skip_group_check=True,
                                )
                    else:
                        at2 = actf.tile([128, 2, V], FP8, tag="at2")
                        for r in range(2):
                            for lo, hi in cbs:
                                absdiff(at2[:, r, lo:hi], s, 2 * k + r, lo, hi)
                        nsplit = (L + MMAX - 1) // MMAX
                        for q in range(nsplit):
                            c0, c1 = q * MMAX, min((q + 1) * MMAX, L)
                            nc.tensor.matmul(
                                P[t][po : po + J, c0:c1],
                                sg2[:, k, :, :],
                                at2[:, :, c0:c1],
                                start=(first and q == 0),
                                stop=(last and q == nsplit - 1),
                                perf_mode=mybir.MatmulPerfMode.DoubleRow,
                                tile_position=(0, po),
                                skip_group_check=True,
                            )
            # partial row sums for tile 0, filled block-wise as its upper
            # blocks arrive (so the final tile's postproc tail is short)
            rs4 = singles.tile([128, 4], FP32)

            def expmax_block(tile_idx, c0, acc):
                # exp in place + max(.,1) + row-sum of one 128-col block
                blk = Ts[tile_idx][:, c0 : c0 + 128]
                nc.scalar.activation(blk, blk, mybir.ActivationFunctionType.Exp)
                nc.vector.tensor_scalar(
                    blk, blk, 1.0, None,
                    op0=mybir.AluOpType.max,
                    op1=mybir.AluOpType.add,
                    accum_out=acc,
                )

            # --- two-stage tile pipeline -------------------------------
            # stage A (at the tile's own boundary): PSUM->SBUF copies and
            # all mirror transposes are DISPATCHED.  stage B (one tile
            # later): diag add, exp, max+rowsum, divide, out DMA.  Every
            # cross-engine latency (xbar transpose ~2.2us, sem hops) gets a
            # full tile-chain of slack, so no engine head-of-line blocks.
            dtmps = {}

            def stage_a(t):
                c0 = 128 * t
                # diag: masked copy (kills PSUM garbage + keeps lower tri)
                nc.vector.tensor_tensor(
                    Ts[t][:, c0 : c0 + 128], P[t][:, c0 : c0 + 128], masks,
                    op=mybir.AluOpType.mult,
                )
                # lower off-diag blocks: plain copies off PSUM, all on
                # ACT (it has slack; DVE is the critical engine)
                for h in range(t):
                    nc.scalar.copy(
                        Ts[t][:, 128 * h : 128 * h + 128],
                        P[t][:, 128 * h : 128 * h + 128],
                    )
                # diag mirror transpose via DMA xbar; the add happens in
                # stage B a tile later
                dtmp = ttmp.tile([128, 128], BF16, tag="dtmp")
                nc.sync.dma_start_transpose(dtmp, Ts[t][:, c0 : c0 + 128])
                dtmps[t] = dtmp
                # provide upper blocks for tiles processed later (t' < t)
                for tp in range(1, t):
                    nc.sync.dma_start_transpose(
                        Ts[tp][:, c0 : c0 + 128],
                        Ts[t][:, 128 * tp : 128 * tp + 128],
                    )
                if t >= 2:
                    nc.sync.dma_start_transpose(
                        Ts[0][:, c0 : c0 + 128], Ts[t][:, 0:128]
                    )
                elif t == 1:
                    # no slack left for the xbar: PE transpose (~150ns)
                    pt0 = pmisc.tile([128, 128], BF16, tag="pt")
                    nc.tensor.transpose(pt0, Ts[1][:, 0:128], ident)
                    nc.scalar.copy(Ts[0][:, c0 : c0 + 128], pt0)
                    expmax_block(0, c0, rs4[:, 1:2])

            def stage_b(t):
                c0 = 128 * t
                nc.vector.tensor_tensor(
                    Ts[t][:, c0 : c0 + 128], Ts[t][:, c0 : c0 + 128],
                    dtmps.pop(t),
                    op=mybir.AluOpType.add,
                )
                rs = small.tile([128, 1], FP32, tag="rs")
                nc.scalar.activation(
                    Ts[t], Ts[t], mybir.ActivationFunctionType.Exp
                )
                nc.vector.tensor_scalar(
                    Ts[t], Ts[t], 1.0, None,
                    op0=mybir.AluOpType.max,
                    op1=mybir.AluOpType.add,
                    accum_out=rs,
                )
                ob = obuf.tile([128, V], FP32, tag="ob")
                nc.gpsimd.tensor_scalar(
                    ob, Ts[t], rs, None, op0=mybir.AluOpType.divide
                )
                # t=1's out DMA goes via ACT so the final tile's DMA never
                # queues behind it on SP
                if t == 1:
                    nc.scalar.dma_start(out=out_d[c0 : c0 + 128, :], in_=ob)
                else:
                    nc.sync.dma_start(out=out_d[c0 : c0 + 128, :], in_=ob)

            def process_final():
                # tile 0 finale: the diag block is exp'd STRAIGHT OUT OF
                # PSUM (no SBUF staging copies needed - the diag is never a
                # transpose source, and ACT reads PSUM cheaper than SBUF),
                # then partial-sum combine, divide, out
                nc.scalar.activation(
                    Ts[0][:, 0:128], P[0][:, 0:128],
                    mybir.ActivationFunctionType.Exp,
                )
                nc.vector.tensor_scalar(
                    Ts[0][:, 0:128], Ts[0][:, 0:128], 1.0, None,
                    op0=mybir.AluOpType.max,
                    op1=mybir.AluOpType.add,
                    accum_out=rs4[:, 0:1],
                )
                rs = small.tile([128, 1], FP32, tag="rs")
                nc.vector.reduce_sum(rs, rs4, axis=mybir.AxisListType.X)
                ob = obuf.tile([128, V], FP32, tag="ob")
                nc.vector.tensor_scalar(
                    ob, Ts[0], rs, None, op0=mybir.AluOpType.divide
                )
                nc.sync.dma_start(out=out_d[0:128, :], in_=ob)

            for pi, (klo, khi) in enumerate(PASSES):
                last = pi == len(PASSES) - 1
                # early passes run ascending (tiny chains first while the
                # xg DMA chunks stream in); the last pass descending so the
                # tile postproc pipeline works t=3..0
                order = range(NSET - 1, -1, -1) if last else range(NSET)
                for s in order:
                    chain(s, klo, khi)
                    if not last:
                        continue
                    # boundary slots (one-chain deferral baked in):
                    if s == 11:
                        stage_a(3)
                    elif s == 7:
                        stage_a(2)
                        stage_b(3)
                    elif s == 4:
                        expmax_block(0, 384, rs4[:, 3:4])
                    elif s == 3:
                        stage_a(1)
                        stage_b(2)
                    elif s == 2:
                        expmax_block(0, 256, rs4[:, 2:3])
                        stage_b(1)
            process_final()
    return nc
